# revision 7
# baseline (speedup 1.0000x reference)
"""Trainium2 Bass kernel for the e3nn-style Clebsch-Gordan tensor product.

Computes, for each batch element z:
    Out[z, u*Do+io (+rowbase_i), v*Di+ji (+colbase_j)]
        = sum_k R[z, roff_ij + (u*16+v)*nlf + k] * Wn[z, k, io, ji]
    Wn[z, k, io, ji] = norm[i,j,z] * sum_m C[k,io,ji,m] * Y[m, z]

Strategy (per NeuronCore, batch is data-parallel over 8 cores):
  - batch (z) lives on SBUF partitions, 128 per tile (4 tiles per core).
  - Wn computed by one PE matmul per z-tile:  psum[z,259] = Y[25,z].T @ C2[25,259],
    evacuated to SBUF with a per-partition norm multiply.
  - The contraction is 259 fused multiply-accumulate ops per z-tile:
        scalar_tensor_tensor(out, R_slice, W_col, out, mult, add)
    with FD = (u,v) = 256 dense elements/partition, split across
    ScalarE (first k-term, activation-copy with per-partition scale),
    VectorE and GPSIMD (accumulating terms).
  - Output granules are full row-blocks (16*Do rows x 144 cols) so the
    store DMA is fully contiguous per partition.
"""

import os
import numpy as np
from math import factorial, sqrt

# ---------------------------------------------------------------- problem dims
BATCH = 4096
N_CORES = 8
ZC = BATCH // N_CORES          # 512 batch elements per core
ZT = 128                       # z-tile = SBUF partitions
NZT = ZC // ZT                 # 4 z-tiles per core
Y_DIM = 25
N_PATH = 4864
RS = [(16, 0), (16, 1), (16, 2)]   # (mul, l) for both in and out
MUL = 16
OUT_DIM = 144

# tuning knobs
USE_GPSIMD = True
# estimated per-instruction ns cost used for static load balancing
COST_ACT_FIRST = 400.0     # (224+256)/1.2
COST_DVE_FIRST = 330.0     # tensor_scalar fp32
COST_GPS_FIRST = 650.0     # q7 tensor_scalar
COST_DVE_STT = 330.0       # (58+256)/0.96
COST_GPS_ACC = 1210.0      # q7 tensor_tensor mult + add (2 ops)
COST_ACT_EVAC = 260.0
COST_DVE_EVAC = 270.0


# ------------------------------------------------------- CG coefficient tables
def _wigner_3j(j1, j2, j3, m1, m2, m3):
    if m1 + m2 + m3 != 0 or not (abs(j1 - j2) <= j3 <= j1 + j2):
        return 0.0
    if abs(m1) > j1 or abs(m2) > j2 or abs(m3) > j3:
        return 0.0
    f = factorial
    pre = sqrt(f(j1 + j2 - j3) * f(j1 - j2 + j3) * f(-j1 + j2 + j3) / f(j1 + j2 + j3 + 1))
    pre *= sqrt(f(j1 + m1) * f(j1 - m1) * f(j2 + m2) * f(j2 - m2) * f(j3 + m3) * f(j3 - m3))
    s = 0.0
    for t in range(max(0, j2 - j3 - m1, j1 - j3 + m2),
                   min(j1 + j2 - j3, j1 - m1, j2 + m2) + 1):
        s += (-1) ** t / (f(t) * f(j3 - j2 + m1 + t) * f(j3 - j1 - m2 + t)
                          * f(j1 + j2 - j3 - t) * f(j1 - m1 - t) * f(j2 + m2 - t))
    return (-1) ** (j1 - j2 - m3) * pre * s


def _real_to_complex(l):
    A = np.zeros((2 * l + 1, 2 * l + 1), dtype=np.complex128)
    A[l, l] = 1.0
    s2 = 1.0 / np.sqrt(2.0)
    for m in range(1, l + 1):
        A[l + m, l + m] = (-1) ** m * s2
        A[l + m, l - m] = 1j * (-1) ** m * s2
        A[l - m, l + m] = s2
        A[l - m, l - m] = -1j * s2
    return A


def _clebsch_gordan(l1, l2, l3):
    T = np.zeros((2 * l1 + 1, 2 * l2 + 1, 2 * l3 + 1))
    for m1 in range(-l1, l1 + 1):
        for m2 in range(-l2, l2 + 1):
            m3 = -(m1 + m2)
            if abs(m3) <= l3:
                T[m1 + l1, m2 + l2, m3 + l3] = _wigner_3j(l1, l2, l3, m1, m2, m3)
    A1, A2, A3 = _real_to_complex(l1), _real_to_complex(l2), _real_to_complex(l3)
    C = np.einsum('abc,ai,bj,ck->ijk', T, A1.conj(), A2.conj(), A3.conj())
    C = C.real if np.linalg.norm(C.real) >= np.linalg.norm(C.imag) else C.imag
    n = np.linalg.norm(C)
    return (C / n).astype(np.float32) if n > 0 else C.astype(np.float32)


SET_LF = sorted({l for (_, li) in RS for (_, lo) in RS
                 for l in range(abs(li - lo), li + lo + 1)})
Y_OFF = {l: sum(2 * k + 1 for k in SET_LF if k < l) for l in SET_LF}


class _Block:
    pass


def _make_blocks():
    blocks = []
    roff = 0
    woff = 0
    rowbases = [0, 16, 64]
    colbases = [0, 16, 64]
    for i, (mo, lo) in enumerate(RS):
        for j, (mi, li) in enumerate(RS):
            b = _Block()
            b.i, b.j, b.lo, b.li = i, j, lo, li
            b.Do, b.Di = 2 * lo + 1, 2 * li + 1
            b.lfs = list(range(abs(li - lo), li + lo + 1))
            b.nlf = len(b.lfs)
            b.roff = roff
            b.woff = woff
            b.wlen = b.nlf * b.Do * b.Di
            b.rowbase = rowbases[i]
            b.colbase = colbases[j]
            b.nidx = 3 * i + j
            roff += mo * mi * b.nlf
            woff += b.wlen
            blocks.append(b)
    return blocks, woff


BLOCKS, W_COLS = _make_blocks()          # W_COLS == 259


def _make_c2():
    C2 = np.zeros((Y_DIM, W_COLS), dtype=np.float32)
    for b in BLOCKS:
        for k, lf in enumerate(b.lfs):
            cg = _clebsch_gordan(b.lo, b.li, lf)        # [Do, Di, 2lf+1]
            o = Y_OFF[lf]
            for io in range(b.Do):
                for ji in range(b.Di):
                    col = b.woff + k * b.Do * b.Di + io * b.Di + ji
                    C2[o:o + 2 * lf + 1, col] = cg[io, ji, :]
    return C2


C2_HOST = _make_c2()


# ------------------------------------------------------------- bass program
_CACHED = {}


def _build_program():
    import concourse.bacc as bacc
    import concourse.mybir as mybir
    import concourse.tile as tile
    from concourse.bass import AP

    f32 = mybir.dt.float32
    ALU = mybir.AluOpType

    def mkap(base, off, dims):
        # strided free-dim view of a tile: dims = [(step, count), ...]
        return AP(base.tensor, base.offset + off,
                  [list(base.ap[0])] + [[s, c] for s, c in dims])

    nc = bacc.Bacc("TRN2", target_bir_lowering=False, debug=False,
                   enable_asserts=False, num_devices=N_CORES)

    Yt = nc.dram_tensor("Y", [Y_DIM, ZC], f32, kind="ExternalInput")
    Rt = nc.dram_tensor("R", [ZC, N_PATH], f32, kind="ExternalInput")
    Nt = nc.dram_tensor("NT", [ZC, 9], f32, kind="ExternalInput")
    Ct = nc.dram_tensor("C2", [Y_DIM, W_COLS], f32, kind="ExternalInput")
    Ot = nc.dram_tensor("O", [ZC, OUT_DIM, OUT_DIM], f32, kind="ExternalOutput")

    # R granule (row-block) slices: blocks (i,0..2) are contiguous in path space
    rg_off = []
    rg_len = []
    for i in range(3):
        bs = [b for b in BLOCKS if b.i == i]
        off = bs[0].roff
        end = bs[-1].roff + MUL * MUL * bs[-1].nlf
        rg_off.append(off)
        rg_len.append(end - off)

    with tile.TileContext(nc) as tc:
        with tc.tile_pool(name="const", bufs=1) as constp, \
             tc.tile_pool(name="wpool", bufs=2) as wpool, \
             tc.tile_pool(name="ypool", bufs=2) as ypool, \
             tc.tile_pool(name="rpool", bufs=3) as rpool, \
             tc.tile_pool(name="gpool", bufs=2) as gpool, \
             tc.tile_pool(name="tpool", bufs=4) as tpool, \
             tc.tile_pool(name="psum", bufs=2, space="PSUM") as psump:

            c2 = constp.tile([Y_DIM, W_COLS], f32)
            nc.sync.dma_start(out=c2[:], in_=Ct.ap())

            # static greedy load balancing across ACT / DVE / GPSIMD
            load = {"act": 0.0, "dve": 0.0, "gps": 0.0}

            def pick(cands):
                e = min(cands, key=lambda kv: load[kv[0]] + kv[1])
                load[e[0]] += e[1]
                return e[0]

            for t in range(NZT):
                z0 = t * ZT
                ysb = ypool.tile([Y_DIM, ZT], f32, tag="y")
                nc.sync.dma_start(out=ysb[:], in_=Yt.ap()[:, z0:z0 + ZT])

                # wn holds [ W (259 cols) | norm (9 cols) ]
                wn = wpool.tile([ZT, W_COLS + 9], f32, tag="wn")
                nc.sync.dma_start(out=wn[:, W_COLS:W_COLS + 9],
                                  in_=Nt.ap()[z0:z0 + ZT, :])

                pw = psump.tile([ZT, W_COLS], f32, tag="pw")
                nc.tensor.matmul(pw[:], ysb[:], c2[:], start=True, stop=True)

                # evacuate PSUM -> SBUF with the per-partition norm multiply
                for b in BLOCKS:
                    ncol = wn[:, W_COLS + b.nidx:W_COLS + b.nidx + 1]
                    e = pick([("act", COST_ACT_EVAC), ("dve", COST_DVE_EVAC)])
                    if e == "act":
                        nc.scalar.mul(wn[:, b.woff:b.woff + b.wlen],
                                      pw[:, b.woff:b.woff + b.wlen], ncol)
                    else:
                        nc.vector.tensor_scalar_mul(
                            wn[:, b.woff:b.woff + b.wlen],
                            pw[:, b.woff:b.woff + b.wlen], ncol)

                for i in range(3):
                    bs = [b for b in BLOCKS if b.i == i]
                    Do = bs[0].Do
                    nrows = MUL * Do

                    rsl = rpool.tile([ZT, rg_len[i]], f32, tag="rg")
                    nc.sync.dma_start(
                        out=rsl[:],
                        in_=Rt.ap()[z0:z0 + ZT, rg_off[i]:rg_off[i] + rg_len[i]])

                    G = gpool.tile([ZT, nrows * OUT_DIM], f32, tag="g")
                    gbase = G[:]
                    rbase = rsl[:]

                    for b in bs:
                        ro = b.roff - rg_off[i]
                        for io in range(b.Do):
                            for ji in range(b.Di):
                                # output slice [z, u(16), v(16)] for this (io, ji)
                                gsl = mkap(gbase,
                                           io * OUT_DIM + b.colbase + ji,
                                           [(b.Do * OUT_DIM, MUL), (b.Di, MUL)])
                                for k in range(b.nlf):
                                    rk = mkap(rbase, ro + k,
                                              [(MUL * b.nlf, MUL), (b.nlf, MUL)])
                                    wcol = b.woff + k * b.Do * b.Di + io * b.Di + ji
                                    wap = wn[:, wcol:wcol + 1]
                                    if k == 0:
                                        cands = [("act", COST_ACT_FIRST),
                                                 ("dve", COST_DVE_FIRST)]
                                        if USE_GPSIMD:
                                            cands.append(("gps", COST_GPS_FIRST))
                                        e = pick(cands)
                                        if e == "act":
                                            nc.scalar.mul(gsl, rk, wap)
                                        elif e == "dve":
                                            nc.vector.tensor_scalar_mul(gsl, rk, wap)
                                        else:
                                            nc.gpsimd.tensor_scalar_mul(gsl, rk, wap)
                                    else:
                                        cands = [("dve", COST_DVE_STT)]
                                        if USE_GPSIMD:
                                            cands.append(("gps", COST_GPS_ACC))
                                        e = pick(cands)
                                        if e == "dve":
                                            nc.vector.scalar_tensor_tensor(
                                                gsl, rk, wap, gsl, ALU.mult, ALU.add)
                                        else:
                                            tmp = tpool.tile([ZT, MUL * MUL], f32,
                                                             tag="t")
                                            tsl = mkap(tmp[:], 0,
                                                       [(MUL, MUL), (1, MUL)])
                                            nc.gpsimd.tensor_scalar_mul(tsl, rk, wap)
                                            nc.gpsimd.tensor_tensor(
                                                out=gsl, in0=gsl, in1=tsl,
                                                op=ALU.add)

                    nc.scalar.dma_start(
                        out=Ot.ap()[z0:z0 + ZT,
                                    bs[0].rowbase:bs[0].rowbase + nrows, :],
                        in_=G[:])

    nc.compile()
    return nc


def _get_program():
    if "nc" not in _CACHED:
        _CACHED["nc"] = _build_program()
    return _CACHED["nc"]


# ------------------------------------------------------------------ entrypoint
def _run(Y, R, norm_coef, **spmd_kwargs):
    from concourse.bass_utils import run_bass_kernel_spmd

    Y = np.ascontiguousarray(np.asarray(Y, dtype=np.float32))
    R = np.ascontiguousarray(np.asarray(R, dtype=np.float32))
    norm_coef = np.asarray(norm_coef, dtype=np.float32)
    normT = np.ascontiguousarray(norm_coef.reshape(9, BATCH).T)  # [4096, 9]

    nc = _get_program()
    in_maps = []
    for c in range(N_CORES):
        z0, z1 = c * ZC, (c + 1) * ZC
        in_maps.append({
            "Y": np.ascontiguousarray(Y[:, z0:z1]),
            "R": np.ascontiguousarray(R[z0:z1]),
            "NT": np.ascontiguousarray(normT[z0:z1]),
            "C2": C2_HOST,
        })
    res = run_bass_kernel_spmd(nc, in_maps, core_ids=list(range(N_CORES)),
                               **spmd_kwargs)
    out = np.concatenate([r["O"] for r in res.results], axis=0)
    return out, res


def kernel(Y, R, norm_coef):
    out, _ = _run(Y, R, norm_coef)
    return out


# revision 14
# speedup vs baseline: 30447.1417x; 30447.1417x over previous
"""Trainium2 Bass kernel for the e3nn-style Clebsch-Gordan tensor product.

Computes, for each batch element z:
    Out[z, u*Do+io (+rowbase_i), v*Di+ji (+colbase_j)]
        = sum_k R[z, roff_ij + (u*16+v)*nlf + k] * Wn[z, k, io, ji]
    Wn[z, k, io, ji] = norm[i,j,z] * sum_m C[k,io,ji,m] * Y[m, z]

Strategy (per NeuronCore, batch is data-parallel over 8 cores):
  - batch (z) lives on SBUF partitions, 128 per tile (4 tiles per core).
  - Wn computed by one PE matmul per z-tile:  psum[z,259] = Y[25,z].T @ C2[25,259],
    evacuated to SBUF with a per-partition norm multiply.
  - The contraction is 259 fused multiply-accumulate ops per z-tile:
        scalar_tensor_tensor(out, R_slice, W_col, out, mult, add)
    with FD = (u,v) = 256 dense elements/partition, split across
    ScalarE (first k-term, activation-copy with per-partition scale),
    VectorE and GPSIMD (accumulating terms).
  - Output granules are full row-blocks (16*Do rows x 144 cols) so the
    store DMA is fully contiguous per partition.
"""

import os
import numpy as np
from math import factorial, sqrt

# ---------------------------------------------------------------- problem dims
BATCH = 4096
N_CORES = 8
ZC = BATCH // N_CORES          # 512 batch elements per core
ZT = 128                       # z-tile = SBUF partitions
NZT = ZC // ZT                 # 4 z-tiles per core
Y_DIM = 25
N_PATH = 4864
RS = [(16, 0), (16, 1), (16, 2)]   # (mul, l) for both in and out
MUL = 16
OUT_DIM = 144

# tuning knobs
USE_GPSIMD = True
# estimated per-instruction ns cost used for static load balancing
COST_ACT_FIRST = 400.0     # (224+256)/1.2
COST_DVE_FIRST = 330.0     # tensor_scalar fp32
COST_GPS_FIRST = 650.0     # q7 tensor_scalar
COST_DVE_STT = 330.0       # (58+256)/0.96
COST_GPS_ACC = 1210.0      # q7 tensor_tensor mult + add (2 ops)
COST_ACT_EVAC = 260.0
COST_DVE_EVAC = 270.0


# ------------------------------------------------------- CG coefficient tables
def _wigner_3j(j1, j2, j3, m1, m2, m3):
    if m1 + m2 + m3 != 0 or not (abs(j1 - j2) <= j3 <= j1 + j2):
        return 0.0
    if abs(m1) > j1 or abs(m2) > j2 or abs(m3) > j3:
        return 0.0
    f = factorial
    pre = sqrt(f(j1 + j2 - j3) * f(j1 - j2 + j3) * f(-j1 + j2 + j3) / f(j1 + j2 + j3 + 1))
    pre *= sqrt(f(j1 + m1) * f(j1 - m1) * f(j2 + m2) * f(j2 - m2) * f(j3 + m3) * f(j3 - m3))
    s = 0.0
    for t in range(max(0, j2 - j3 - m1, j1 - j3 + m2),
                   min(j1 + j2 - j3, j1 - m1, j2 + m2) + 1):
        s += (-1) ** t / (f(t) * f(j3 - j2 + m1 + t) * f(j3 - j1 - m2 + t)
                          * f(j1 + j2 - j3 - t) * f(j1 - m1 - t) * f(j2 + m2 - t))
    return (-1) ** (j1 - j2 - m3) * pre * s


def _real_to_complex(l):
    A = np.zeros((2 * l + 1, 2 * l + 1), dtype=np.complex128)
    A[l, l] = 1.0
    s2 = 1.0 / np.sqrt(2.0)
    for m in range(1, l + 1):
        A[l + m, l + m] = (-1) ** m * s2
        A[l + m, l - m] = 1j * (-1) ** m * s2
        A[l - m, l + m] = s2
        A[l - m, l - m] = -1j * s2
    return A


def _clebsch_gordan(l1, l2, l3):
    T = np.zeros((2 * l1 + 1, 2 * l2 + 1, 2 * l3 + 1))
    for m1 in range(-l1, l1 + 1):
        for m2 in range(-l2, l2 + 1):
            m3 = -(m1 + m2)
            if abs(m3) <= l3:
                T[m1 + l1, m2 + l2, m3 + l3] = _wigner_3j(l1, l2, l3, m1, m2, m3)
    A1, A2, A3 = _real_to_complex(l1), _real_to_complex(l2), _real_to_complex(l3)
    C = np.einsum('abc,ai,bj,ck->ijk', T, A1.conj(), A2.conj(), A3.conj())
    C = C.real if np.linalg.norm(C.real) >= np.linalg.norm(C.imag) else C.imag
    n = np.linalg.norm(C)
    return (C / n).astype(np.float32) if n > 0 else C.astype(np.float32)


SET_LF = sorted({l for (_, li) in RS for (_, lo) in RS
                 for l in range(abs(li - lo), li + lo + 1)})
Y_OFF = {l: sum(2 * k + 1 for k in SET_LF if k < l) for l in SET_LF}


class _Block:
    pass


def _make_blocks():
    blocks = []
    roff = 0
    woff = 0
    rowbases = [0, 16, 64]
    colbases = [0, 16, 64]
    for i, (mo, lo) in enumerate(RS):
        for j, (mi, li) in enumerate(RS):
            b = _Block()
            b.i, b.j, b.lo, b.li = i, j, lo, li
            b.Do, b.Di = 2 * lo + 1, 2 * li + 1
            b.lfs = list(range(abs(li - lo), li + lo + 1))
            b.nlf = len(b.lfs)
            b.roff = roff
            b.woff = woff
            b.wlen = b.nlf * b.Do * b.Di
            b.rowbase = rowbases[i]
            b.colbase = colbases[j]
            b.nidx = 3 * i + j
            roff += mo * mi * b.nlf
            woff += b.wlen
            blocks.append(b)
    return blocks, woff


BLOCKS, W_COLS = _make_blocks()          # W_COLS == 259


def _make_c2():
    C2 = np.zeros((Y_DIM, W_COLS), dtype=np.float32)
    for b in BLOCKS:
        for k, lf in enumerate(b.lfs):
            cg = _clebsch_gordan(b.lo, b.li, lf)        # [Do, Di, 2lf+1]
            o = Y_OFF[lf]
            for io in range(b.Do):
                for ji in range(b.Di):
                    col = b.woff + k * b.Do * b.Di + io * b.Di + ji
                    C2[o:o + 2 * lf + 1, col] = cg[io, ji, :]
    return C2


C2_HOST = _make_c2()


# ------------------------------------------------------------- bass program
_CACHED = {}


def _build_program():
    import concourse.bacc as bacc
    import concourse.mybir as mybir
    import concourse.tile as tile
    from concourse.bass import AP

    f32 = mybir.dt.float32
    ALU = mybir.AluOpType

    def mkap(base, off, dims):
        # strided free-dim view of a tile: dims = [(step, count), ...]
        return AP(base.tensor, base.offset + off,
                  [list(base.ap[0])] + [[s, c] for s, c in dims])

    nc = bacc.Bacc("TRN2", target_bir_lowering=False, debug=False,
                   enable_asserts=False, num_devices=N_CORES)

    Yt = nc.dram_tensor("Y", [Y_DIM, ZC], f32, kind="ExternalInput")
    Rt = nc.dram_tensor("R", [ZC, N_PATH], f32, kind="ExternalInput")
    Nt = nc.dram_tensor("NT", [ZC, 9], f32, kind="ExternalInput")
    Ct = nc.dram_tensor("C2", [Y_DIM, W_COLS], f32, kind="ExternalInput")
    Ot = nc.dram_tensor("O", [ZC, OUT_DIM, OUT_DIM], f32, kind="ExternalOutput")

    # R granule (row-block) slices: blocks (i,0..2) are contiguous in path space
    rg_off = []
    rg_len = []
    for i in range(3):
        bs = [b for b in BLOCKS if b.i == i]
        off = bs[0].roff
        end = bs[-1].roff + MUL * MUL * bs[-1].nlf
        rg_off.append(off)
        rg_len.append(end - off)

    with tile.TileContext(nc) as tc:
        with tc.tile_pool(name="const", bufs=1) as constp, \
             tc.tile_pool(name="wpool", bufs=2) as wpool, \
             tc.tile_pool(name="ypool", bufs=2) as ypool, \
             tc.tile_pool(name="rpool", bufs=3) as rpool, \
             tc.tile_pool(name="gpool", bufs=2) as gpool, \
             tc.tile_pool(name="tpool", bufs=4) as tpool, \
             tc.tile_pool(name="psum", bufs=2, space="PSUM") as psump:

            c2 = constp.tile([Y_DIM, W_COLS], f32)
            nc.sync.dma_start(out=c2[:], in_=Ct.ap())

            # static greedy load balancing across ACT / DVE / GPSIMD
            load = {"act": 0.0, "dve": 0.0, "gps": 0.0}

            def pick(cands):
                e = min(cands, key=lambda kv: load[kv[0]] + kv[1])
                load[e[0]] += e[1]
                return e[0]

            for t in range(NZT):
                z0 = t * ZT
                ysb = ypool.tile([Y_DIM, ZT], f32, tag="y")
                nc.sync.dma_start(out=ysb[:], in_=Yt.ap()[:, z0:z0 + ZT])

                # wn holds [ W (259 cols) | norm (9 cols) ]
                wn = wpool.tile([ZT, W_COLS + 9], f32, tag="wn")
                nc.sync.dma_start(out=wn[:, W_COLS:W_COLS + 9],
                                  in_=Nt.ap()[z0:z0 + ZT, :])

                pw = psump.tile([ZT, W_COLS], f32, tag="pw")
                nc.tensor.matmul(pw[:], ysb[:], c2[:], start=True, stop=True)

                # evacuate PSUM -> SBUF with the per-partition norm multiply
                for b in BLOCKS:
                    ncol = wn[:, W_COLS + b.nidx:W_COLS + b.nidx + 1]
                    e = pick([("act", COST_ACT_EVAC), ("dve", COST_DVE_EVAC)])
                    if e == "act":
                        nc.scalar.mul(wn[:, b.woff:b.woff + b.wlen],
                                      pw[:, b.woff:b.woff + b.wlen], ncol)
                    else:
                        nc.vector.tensor_scalar_mul(
                            wn[:, b.woff:b.woff + b.wlen],
                            pw[:, b.woff:b.woff + b.wlen], ncol)

                for i in range(3):
                    bs = [b for b in BLOCKS if b.i == i]
                    Do = bs[0].Do
                    nrows = MUL * Do

                    rsl = rpool.tile([ZT, rg_len[i]], f32, tag="rg")
                    nc.sync.dma_start(
                        out=rsl[:],
                        in_=Rt.ap()[z0:z0 + ZT, rg_off[i]:rg_off[i] + rg_len[i]])

                    G = gpool.tile([ZT, nrows * OUT_DIM], f32, tag="g")
                    gbase = G[:]
                    rbase = rsl[:]

                    for b in bs:
                        ro = b.roff - rg_off[i]
                        for io in range(b.Do):
                            for ji in range(b.Di):
                                # output slice [z, u(16), v(16)] for this (io, ji)
                                gsl = mkap(gbase,
                                           io * OUT_DIM + b.colbase + ji,
                                           [(b.Do * OUT_DIM, MUL), (b.Di, MUL)])
                                for k in range(b.nlf):
                                    rk = mkap(rbase, ro + k,
                                              [(MUL * b.nlf, MUL), (b.nlf, MUL)])
                                    wcol = b.woff + k * b.Do * b.Di + io * b.Di + ji
                                    wap = wn[:, wcol:wcol + 1]
                                    if k == 0:
                                        cands = [("act", COST_ACT_FIRST),
                                                 ("dve", COST_DVE_FIRST)]
                                        if USE_GPSIMD:
                                            cands.append(("gps", COST_GPS_FIRST))
                                        e = pick(cands)
                                        if e == "act":
                                            nc.scalar.mul(gsl, rk, wap)
                                        elif e == "dve":
                                            nc.vector.tensor_scalar_mul(gsl, rk, wap)
                                        else:
                                            nc.gpsimd.tensor_scalar_mul(gsl, rk, wap)
                                    else:
                                        cands = [("dve", COST_DVE_STT)]
                                        if USE_GPSIMD:
                                            cands.append(("gps", COST_GPS_ACC))
                                        e = pick(cands)
                                        if e == "dve":
                                            nc.vector.scalar_tensor_tensor(
                                                gsl, rk, wap, gsl, ALU.mult, ALU.add)
                                        else:
                                            tmp = tpool.tile([ZT, MUL * MUL], f32,
                                                             tag="t")
                                            tsl = mkap(tmp[:], 0,
                                                       [(MUL, MUL), (1, MUL)])
                                            nc.gpsimd.tensor_scalar_mul(tsl, rk, wap)
                                            nc.gpsimd.tensor_tensor(
                                                out=gsl, in0=gsl, in1=tsl,
                                                op=ALU.add)

                    nc.scalar.dma_start(
                        out=Ot.ap()[z0:z0 + ZT,
                                    bs[0].rowbase:bs[0].rowbase + nrows, :],
                        in_=G[:])

    nc.compile()
    return nc


def _get_program():
    if "nc" not in _CACHED:
        _CACHED["nc"] = _build_program()
    return _CACHED["nc"]


# ------------------------------------------------------------------ entrypoint
def _run(Y, R, norm_coef, **spmd_kwargs):
    from concourse.bass_utils import run_bass_kernel_spmd

    Y = np.ascontiguousarray(np.asarray(Y, dtype=np.float32))
    R = np.ascontiguousarray(np.asarray(R, dtype=np.float32))
    norm_coef = np.asarray(norm_coef, dtype=np.float32)
    normT = np.ascontiguousarray(norm_coef.reshape(9, BATCH).T)  # [4096, 9]

    nc = _get_program()
    in_maps = []
    for c in range(N_CORES):
        z0, z1 = c * ZC, (c + 1) * ZC
        in_maps.append({
            "Y": np.ascontiguousarray(Y[:, z0:z1]),
            "R": np.ascontiguousarray(R[z0:z1]),
            "NT": np.ascontiguousarray(normT[z0:z1]),
            "C2": C2_HOST,
        })
    res = run_bass_kernel_spmd(nc, in_maps, core_ids=list(range(N_CORES)),
                               **spmd_kwargs)
    out = np.concatenate([r["O"] for r in res.results], axis=0)
    return out, res


def kernel(Y, R, norm_coef):
    out, _ = _run(Y, R, norm_coef)
    return out


# ------------------------------------------------- timing helper (test only)
def _timed_run(Y, R, norm_coef, iters=10):
    """Estimate per-execution device time by running a chain of `iters`
    NEFF executions on-device (outputs fed back as the donated output
    operands of the next call) and differencing two chain lengths."""
    import time
    import jax
    import concourse.mybir as mybir
    from concourse.bass2jax import _bass_exec_p, install_neuronx_cc_hook
    from jax.sharding import Mesh, PartitionSpec
    from jax.experimental.shard_map import shard_map

    from concourse.bass2jax import partition_id_tensor

    install_neuronx_cc_hook()
    nc = _get_program()
    pname = nc.partition_id_tensor.name if nc.partition_id_tensor else None

    Y = np.ascontiguousarray(np.asarray(Y, dtype=np.float32))
    R = np.ascontiguousarray(np.asarray(R, dtype=np.float32))
    norm_coef = np.asarray(norm_coef, dtype=np.float32)
    normT = np.ascontiguousarray(norm_coef.reshape(9, BATCH).T)

    in_names, out_names, out_avals, zero_outs = [], [], [], []
    for alloc in nc.m.functions[0].allocations:
        if not isinstance(alloc, mybir.MemoryLocationSet):
            continue
        name = alloc.memorylocations[0].name
        if alloc.kind == "ExternalInput":
            if name != pname:
                in_names.append(name)
        elif alloc.kind == "ExternalOutput":
            out_names.append(name)
            shape = tuple(alloc.tensor_shape)
            dtype = mybir.dt.np(alloc.dtype)
            out_avals.append(jax.core.ShapedArray(shape, dtype))
            zero_outs.append(np.zeros(shape, dtype))
    n_params = len(in_names)
    all_names = tuple(in_names + out_names + ([pname] if pname else []))

    import jax.numpy as jnp
    from jax.sharding import NamedSharding

    def _body(*args):
        operands = list(args)
        if pname is not None:
            operands.append(partition_id_tensor())
        return tuple(_bass_exec_p.bind(
            *operands,
            out_avals=tuple(out_avals),
            in_names=all_names,
            out_names=tuple(out_names),
            lowering_input_output_aliases=(),
            sim_require_finite=True,
            sim_require_nnan=True,
            nc=nc,
        ))

    devices = jax.devices()[:N_CORES]
    mesh = Mesh(np.asarray(devices), ("core",))
    donate = tuple(range(n_params, n_params + len(out_names)))
    shard = NamedSharding(mesh, PartitionSpec("core"))

    fn = jax.jit(
        shard_map(_body, mesh=mesh,
                  in_specs=(PartitionSpec("core"),) * (n_params + len(out_names)),
                  out_specs=(PartitionSpec("core"),) * len(out_names),
                  check_rep=False),
        donate_argnums=donate, keep_unused=True)

    zshapes = [(N_CORES * z.shape[0], *z.shape[1:]) for z in zero_outs]
    zeros_fn = jax.jit(
        lambda: tuple(jnp.zeros(s, z.dtype) for s, z in zip(zshapes, zero_outs)),
        out_shardings=tuple(shard for _ in zero_outs))

    vals = []
    for name in in_names:
        if name == "Y":
            vals.append(np.concatenate(
                [Y[:, c * ZC:(c + 1) * ZC] for c in range(N_CORES)], axis=0))
        elif name == "R":
            vals.append(R)
        elif name == "NT":
            vals.append(normT)
        elif name == "C2":
            vals.append(np.concatenate([C2_HOST] * N_CORES, axis=0))
        else:
            raise KeyError(name)
    cin = [jax.device_put(v, shard) for v in vals]
    jax.block_until_ready(cin)

    def run_k(k, with_exec=True):
        outs = None
        t0 = time.perf_counter()
        for _ in range(k):
            zs = zeros_fn()
            if with_exec:
                outs = fn(*cin, *zs)
            else:
                outs = zs
        jax.block_until_ready(outs)
        return time.perf_counter() - t0

    run_k(2)  # compile + warm
    run_k(2, with_exec=False)
    k_lo, k_hi = 2, 2 + iters
    t_lo = min(run_k(k_lo) for _ in range(3))
    t_hi = min(run_k(k_hi) for _ in range(3))
    tz_lo = min(run_k(k_lo, False) for _ in range(3))
    tz_hi = min(run_k(k_hi, False) for _ in range(3))
    per_exec = ((t_hi - t_lo) - (tz_hi - tz_lo)) / iters
    return per_exec * 1e9, {"t": (t_lo, t_hi), "tz": (tz_lo, tz_hi)}
